# revision 2
# baseline (speedup 1.0000x reference)
"""Trainium2 Bass kernel for a CRF layer (dense matmul potentials + Viterbi decode).

Contract: kernel(**inputs) takes the FULL unsharded inputs (numpy) and returns
(potentials, decoded_onehot), both [64, 512, 128] float32, matching reference().

Strategy (data-parallel over batch, 8 sequences per NeuronCore, SPMD on 8 cores):
  Phase 1 (TensorE): potentials = x @ kernel + bias + boundary energies, computed
    twice-laid-out: potT [v, token] for the scan, pot [token, v] for the output.
    Boundary/bias adds are folded into the matmul as extra contraction rows.
  Phase 2 (forward Viterbi): replicated-slab layout. Partition p = b*16+vh owns
    the 8 next-states v = vh*8..vh*8+8 of sequence b. Each step:
      - rotation all-gather of the state vector within each 16-partition group
        via 4 stream_shuffles (order seen by partition p: u = (8*vh + j) mod 128)
      - scores = chain_perm + state_row (broadcast over vl)  [128, 8, 128]
      - colmax = reduce_max over u                            [128, 8]
      - backpointers via one max_index over the flat row; decoded to absolute
        u with (idx + 8*vh) mod 128; masked (padding keeps identity bp)
      - state = where(mask, colmax + pot_t, state)
  Phase 3 (backward trace): tag_{t-1} = bp_t[tag_t] extracted with a fused
    (pi_table == tag) * bp_row -> sum  scalar_tensor_tensor per step; one-hot
    rows emitted on the fly and DMA-flushed to DRAM in chunks.
"""

import os
import sys

import numpy as np

sys.path.insert(0, "/opt/trn_rl_repo")

from contextlib import ExitStack

import concourse.bacc as bacc
import concourse.bass as bass
import concourse.mybir as mybir
import concourse.tile as tile

B, T, D, U = 64, 512, 1024, 128
NCORES = 8
BL = B // NCORES          # sequences per core
VH, VL = 16, 8            # partition groups of 16; 8 states per partition
TOK = BL * T              # tokens per core
KC = D // 128             # contraction chunks
TCH = 512                 # phase-1 token chunk
CH = 32                   # backward chunk (rows per bp all-gather / one-hot flush)

F32 = mybir.dt.float32
U16 = mybir.dt.uint16
U8 = mybir.dt.uint8
ALU = mybir.AluOpType
AX = mybir.AxisListType


def _shuffle_mask(s):
    # quadrant-local: keep the 16-group bit, rotate within the group by s
    return [(i & 16) | ((i + s) & 15) for i in range(32)]


def build_module(t_steps=T, tok=TOK, n_devices=NCORES, debug_dump=False):
    """Build the SPMD Bass module. t_steps/tok shrinkable for simulation."""
    nc = bacc.Bacc(
        "TRN2", target_bir_lowering=False, debug=False, num_devices=n_devices
    )

    tch = min(TCH, tok)
    n_tch = tok // tch
    ch = min(CH, t_steps)
    n_ch = t_steps // ch
    assert t_steps % ch == 0 and tok % tch == 0 and tch % 128 == 0

    # ---- DRAM I/O ----
    x_tokT = nc.dram_tensor("x_tokT", [D, tok], F32, kind="ExternalInput")
    w_sb_h = nc.dram_tensor("w_chunks", [128, KC * 128], F32, kind="ExternalInput")
    aug_uv = nc.dram_tensor("aug_uv", [3, 128], F32, kind="ExternalInput")
    aug_tok = nc.dram_tensor("aug_tok", [3, tok], F32, kind="ExternalInput")
    chain_perm = nc.dram_tensor("chain_perm", [128, VL * 128], F32, kind="ExternalInput")
    pi_tab = nc.dram_tensor("pi_tab", [128, 128], F32, kind="ExternalInput")
    u_iota = nc.dram_tensor("u_iota", [128, 128], F32, kind="ExternalInput")
    v_iota = nc.dram_tensor("v_iota", [128, VL], F32, kind="ExternalInput")
    vh8 = nc.dram_tensor("vh8", [128, 1], F32, kind="ExternalInput")
    bpc1 = nc.dram_tensor("bpc1", [128, VL], U16, kind="ExternalInput")
    vh8u = nc.dram_tensor("vh8u", [128, 1], U16, kind="ExternalInput")
    mask_rep = nc.dram_tensor("mask_rep", [128, t_steps], U8, kind="ExternalInput")

    if debug_dump:
        dbg_bp = nc.dram_tensor("dbg_bp", [128, t_steps * VL], F32,
                                kind="ExternalOutput")
        dbg_row = nc.dram_tensor("dbg_row", [128, 128], F32, kind="ExternalOutput")
        dbg_state = nc.dram_tensor("dbg_state", [128, VL * t_steps], F32,
                                   kind="ExternalOutput")
        dbg_sc1 = nc.dram_tensor("dbg_sc1", [128, VL * 128], F32,
                                 kind="ExternalOutput")
        dbg_cmax1 = nc.dram_tensor("dbg_cmax1", [128, VL], F32,
                                   kind="ExternalOutput")
        dbg_idx1 = nc.dram_tensor("dbg_idx1", [128, VL], U16,
                                  kind="ExternalOutput")
    out_pot = nc.dram_tensor("out_pot", [tok, U], F32, kind="ExternalOutput")
    out_oh = nc.dram_tensor("out_oh", [tok, U], F32, kind="ExternalOutput")
    potT_dram = nc.dram_tensor("potT_scratch", [U, tok], F32, kind="Internal")

    with tile.TileContext(nc) as tc, ExitStack() as ctx:
        persist = ctx.enter_context(tc.tile_pool(name="persist", bufs=1))

        # ---- persistent SBUF tiles ----
        w_sb = persist.tile([128, KC * 128], F32, tag="w_sb")
        aug_uv_sb = persist.tile([3, 128], F32, tag="aug_uv")
        aug_tok_sb = persist.tile([3, tok], F32, tag="aug_tok")
        chain_sb = persist.tile([128, VL, 128], F32, tag="chain")
        pi_sb = persist.tile([128, 128], F32, tag="pi")
        uio_sb = persist.tile([128, 128], F32, tag="uio")
        vio_sb = persist.tile([128, VL], F32, tag="vio")
        vh8_sb = persist.tile([128, 1], F32, tag="vh8")
        bpc1_sb = persist.tile([128, VL], U16, tag="bpc1")
        vh8u_sb = persist.tile([128, 1], U16, tag="vh8u")
        mask_sb = persist.tile([128, t_steps], U8, tag="mask")
        potT_sb = persist.tile([128, tok], F32, tag="potT")
        pot_rep = persist.tile([128, VL, t_steps], F32, tag="pot_rep")
        state_row = persist.tile([128, 128], F32, tag="state_row")
        bp_store = persist.tile([128, t_steps, VL], F32, tag="bp_store")

        nc.sync.dma_start(w_sb[:], w_sb_h.ap())
        nc.sync.dma_start(aug_uv_sb[:], aug_uv.ap())
        nc.sync.dma_start(aug_tok_sb[:], aug_tok.ap())
        nc.sync.dma_start(chain_sb[:].rearrange("p a b -> p (a b)"), chain_perm.ap())
        nc.sync.dma_start(pi_sb[:], pi_tab.ap())
        nc.sync.dma_start(uio_sb[:], u_iota.ap())
        nc.sync.dma_start(vio_sb[:], v_iota.ap())
        nc.sync.dma_start(vh8_sb[:], vh8.ap())
        nc.sync.dma_start(bpc1_sb[:], bpc1.ap())
        nc.sync.dma_start(vh8u_sb[:], vh8u.ap())
        nc.sync.dma_start(mask_sb[:], mask_rep.ap())

        # ================= Phase 1: potentials matmuls =================
        with tc.tile_pool(name="ph1", bufs=2) as ph1, \
             tc.tile_pool(name="psA", bufs=2, space="PSUM") as psA_pool, \
             tc.tile_pool(name="psB", bufs=2, space="PSUM") as psB_pool:
            for tc_i in range(n_tch):
                t0 = tc_i * tch
                xT = ph1.tile([128, KC, tch], F32, tag="xT")
                for k in range(KC):
                    nc.sync.dma_start(
                        xT[:, k, :],
                        bass.AP(x_tokT, k * 128 * tok + t0, [[tok, 128], [1, tch]]),
                    )
                # potT[v, tok] += sum_k w[k]^T x[k]  (+ bias/boundary rows)
                psA = psA_pool.tile([128, tch], F32, tag="psA")
                for k in range(KC):
                    nc.tensor.matmul(
                        psA[:], w_sb[:, k * 128:(k + 1) * 128], xT[:, k, :],
                        start=(k == 0), stop=False,
                    )
                nc.tensor.matmul(
                    psA[:], aug_uv_sb[:], aug_tok_sb[:, t0:t0 + tch],
                    start=False, stop=True,
                )
                nc.vector.tensor_copy(potT_sb[:, t0:t0 + tch], psA[:])
                # pot[tok, v]: same sums, output transposed for the DRAM output
                for s in range(tch // 128):
                    s0 = t0 + s * 128
                    psB = psB_pool.tile([128, 128], F32, tag="psB")
                    for k in range(KC):
                        nc.tensor.matmul(
                            psB[:], xT[:, k, s * 128:(s + 1) * 128],
                            w_sb[:, k * 128:(k + 1) * 128],
                            start=(k == 0), stop=False,
                        )
                    nc.tensor.matmul(
                        psB[:], aug_tok_sb[:, s0:s0 + 128], aug_uv_sb[:],
                        start=False, stop=True,
                    )
                    po = ph1.tile([128, 128], F32, tag="po")
                    nc.vector.tensor_copy(po[:], psB[:])
                    nc.sync.dma_start(
                        bass.AP(out_pot, s0 * U, [[U, 128], [1, U]]), po[:]
                    )

        # potT -> DRAM -> pot_rep[(b,vh), vl, t] = pot[b, t, 8*vh+vl]
        nc.sync.dma_start(potT_dram.ap(), potT_sb[:])
        for b in range(BL):
            nc.sync.dma_start(
                pot_rep[b * VH:(b + 1) * VH, :, :],
                bass.AP(potT_dram, b * t_steps,
                        [[VL * tok, VH], [tok, VL], [1, t_steps]]),
            )

        # ================= Phase 2: forward Viterbi =================
        # bp init = identity (padding rows keep it); state init = potentials[:, 0]
        nc.vector.tensor_copy(
            bp_store[:],
            vio_sb[:].unsqueeze(1).broadcast_to([128, t_steps, VL]),
        )
        nc.vector.tensor_copy(state_row[:, 0:VL], pot_rep[:, :, 0])

        masks = {s: _shuffle_mask(s) for s in (1, 2, 4, 8)}
        fwd = ctx.enter_context(tc.tile_pool(name="fwd", bufs=3))
        for t in range(1, t_steps):
            for s in (1, 2, 4, 8):
                nc.vector.stream_shuffle(
                    state_row[:, 8 * s:16 * s], state_row[:, 0:8 * s], masks[s]
                )
            sc = fwd.tile([128, VL, 128], F32, tag="sc")
            nc.vector.tensor_tensor(
                sc[:], chain_sb[:],
                state_row[:].unsqueeze(1).broadcast_to([128, VL, 128]),
                ALU.add,
            )
            cmax = fwd.tile([128, VL], F32, tag="cmax")
            nc.vector.tensor_reduce(cmax[:], sc[:], AX.X, ALU.max)
            idx = fwd.tile([128, VL], U16, tag="idx")
            nc.vector.max_index(idx[:], cmax[:], sc[:].rearrange("p a b -> p (a b)"))
            if debug_dump and t == 1:
                nc.sync.dma_start(dbg_sc1.ap(), sc[:].rearrange("p a b -> p (a b)"))
                nc.sync.dma_start(dbg_cmax1.ap(), cmax[:])
                nc.sync.dma_start(dbg_idx1.ap(), idx[:])
            # state update (masked)
            mbc = mask_sb[:, t:t + 1].broadcast_to([128, VL])
            tmp = fwd.tile([128, VL], F32, tag="tmp")
            nc.vector.tensor_tensor(tmp[:], cmax[:], pot_rep[:, :, t], ALU.add)
            nc.vector.copy_predicated(state_row[:, 0:VL], mbc, tmp[:])
            # bp decode: absolute u = ((idx - 128*vl) + 8*vh) & 127, in u16
            # arithmetic staged to avoid wraparound (HW u16 ALU saturates).
            idf = fwd.tile([128, VL], U16, tag="idf")
            nc.vector.tensor_tensor(idf[:], idx[:], bpc1_sb[:], ALU.subtract)
            idg = fwd.tile([128, VL], U16, tag="idg")
            nc.vector.tensor_tensor(
                idg[:], idf[:], vh8u_sb[:, 0:1].broadcast_to([128, VL]), ALU.add
            )
            idm = fwd.tile([128, VL], U16, tag="idm")
            nc.vector.tensor_scalar(idm[:], idg[:], 127, None, ALU.bitwise_and)
            bpf = fwd.tile([128, VL], F32, tag="bpf")
            nc.vector.tensor_copy(bpf[:], idm[:])
            nc.vector.copy_predicated(bp_store[:, t, :], mbc, bpf[:])

        # ================= Phase 3: backward trace + one-hot =================
        for s in (1, 2, 4, 8):
            nc.vector.stream_shuffle(
                state_row[:, 8 * s:16 * s], state_row[:, 0:8 * s], masks[s]
            )
        if debug_dump:
            nc.sync.dma_start(dbg_bp.ap(),
                              bp_store[:].rearrange("p t v -> p (t v)"))
            nc.sync.dma_start(dbg_row.ap(), state_row[:])
            nc.sync.dma_start(dbg_state.ap(),
                              pot_rep[:].rearrange("p v t -> p (v t)"))
        bwd = ctx.enter_context(tc.tile_pool(name="bwd", bufs=2))
        top8 = bwd.tile([128, 8], F32, tag="top8")
        nc.vector.max(top8[:], state_row[:])
        idx8 = bwd.tile([128, 8], U16, tag="idx8")
        nc.vector.max_index(idx8[:], top8[:], state_row[:])
        lt1 = bwd.tile([128, 1], U16, tag="lt1")
        nc.vector.tensor_tensor(lt1[:], idx8[:, 0:1], vh8u_sb[:], ALU.add)
        lt2 = bwd.tile([128, 1], U16, tag="lt2")
        nc.vector.tensor_scalar(lt2[:], lt1[:], 127, None, ALU.bitwise_and)
        tag_prev = bwd.tile([128, 1], F32, tag="tag")
        nc.vector.tensor_copy(tag_prev[:], lt2[:])

        ring_pool = ctx.enter_context(tc.tile_pool(name="ring", bufs=2))
        oh_pool = ctx.enter_context(tc.tile_pool(name="ohr", bufs=2))
        sel_pool = ctx.enter_context(tc.tile_pool(name="sel", bufs=3))

        oh_ring = oh_pool.tile([128, ch, 128], F32, tag="ohring")
        nc.vector.tensor_scalar(
            oh_ring[:, ch - 1, :], uio_sb[:], tag_prev[:, 0:1], None, ALU.is_equal
        )

        for c in range(n_ch - 1, -1, -1):
            tbase = c * ch
            ring = ring_pool.tile([128, ch, 128], F32, tag="bpring")
            nc.vector.tensor_copy(ring[:, :, 0:VL], bp_store[:, tbase:tbase + ch, :])
            for s in (1, 2, 4, 8):
                nc.vector.stream_shuffle(
                    ring[:, :, 8 * s:16 * s], ring[:, :, 0:8 * s], masks[s]
                )
            t_lo = max(tbase, 1)
            for t in range(tbase + ch - 1, t_lo - 1, -1):
                sel = sel_pool.tile([128, 128], F32, tag="sel")
                tag_new = sel_pool.tile([128, 1], F32, tag="tagn")
                nc.vector.scalar_tensor_tensor(
                    sel[:], pi_sb[:], tag_prev[:, 0:1], ring[:, t - tbase, :],
                    ALU.is_equal, ALU.mult, accum_out=tag_new[:],
                )
                # one-hot row for tag_{t-1}
                r = t - 1
                if r % ch == ch - 1:
                    oh_ring = oh_pool.tile([128, ch, 128], F32, tag="ohring")
                nc.vector.tensor_scalar(
                    oh_ring[:, r % ch, :], uio_sb[:], tag_new[:, 0:1], None,
                    ALU.is_equal,
                )
                tag_prev = tag_new
                if r % ch == 0:
                    nc.sync.dma_start(
                        bass.AP(out_oh, r * U, [[t_steps * U, BL], [1, ch * U]]),
                        oh_ring[0:128:VH, :, :].rearrange("p t v -> p (t v)"),
                    )

    nc.compile()
    if not nc.is_finalized():
        nc.finalize()
    return nc


def _host_prep(inputs, mask, kern, bias, chain_kernel, left_b, right_b, t_steps=T):
    """Build per-core input maps (all numpy, float32)."""
    tok = BL * t_steps
    p = np.arange(128)
    vh = p % VH
    j = np.arange(128)
    # pi[p, j] = (8*vh + j) mod 128 : state/bp all-gather order per partition
    pi = (8 * vh[:, None] + j[None, :]) % 128
    v_of_p = vh[:, None] * VL + np.arange(VL)[None, :]  # [128, VL]

    chain_pp = np.empty((128, VL, 128), np.float32)
    for pp in range(128):
        chain_pp[pp] = chain_kernel[pi[pp]][:, v_of_p[pp]].T  # [VL, 128]

    w_chunks = kern.reshape(KC, 128, 128).transpose(1, 0, 2).reshape(128, KC * 128)
    aug_uv = np.stack([bias, left_b, right_b]).astype(np.float32)

    lengths = mask.sum(axis=1).astype(np.int64)
    n_cores = inputs.shape[0] // BL
    in_maps = []
    for c in range(n_cores):
        bs = c * BL
        xl = inputs[bs:bs + BL].reshape(tok, D)
        ones = np.ones(tok, np.float32)
        start01 = np.zeros((BL, t_steps), np.float32)
        end01 = np.zeros((BL, t_steps), np.float32)
        start01[:, 0] = 1.0
        for b in range(BL):
            end01[b, lengths[bs + b] - 1] = 1.0
        m = {
            "x_tokT": np.ascontiguousarray(xl.T),
            "w_chunks": np.ascontiguousarray(w_chunks),
            "aug_uv": np.ascontiguousarray(aug_uv),
            "aug_tok": np.ascontiguousarray(
                np.stack([ones, start01.ravel(), end01.ravel()])),
            "chain_perm": np.ascontiguousarray(chain_pp.reshape(128, VL * 128)),
            "pi_tab": pi.astype(np.float32),
            "u_iota": np.tile(j.astype(np.float32), (128, 1)),
            "v_iota": v_of_p.astype(np.float32),
            "vh8": (8.0 * vh[:, None]).astype(np.float32),
            "bpc1": np.tile((128 * np.arange(VL, dtype=np.uint16))[None, :],
                            (128, 1)),
            "vh8u": (8 * vh[:, None]).astype(np.uint16),
            "mask_rep": mask[bs + p // VH].astype(np.uint8),
        }
        in_maps.append(m)
    return in_maps


_NC_CACHE = {}


def kernel(inputs, mask, kernel, bias, chain_kernel, left_boundary, right_boundary):
    inputs = np.asarray(inputs, np.float32)
    mask_np = np.asarray(mask)
    kern = np.asarray(kernel, np.float32)
    bias = np.asarray(bias, np.float32)
    chain = np.asarray(chain_kernel, np.float32)
    lb = np.asarray(left_boundary, np.float32)
    rb = np.asarray(right_boundary, np.float32)

    from concourse.bass_utils import run_bass_kernel_spmd

    if "nc" not in _NC_CACHE:
        _NC_CACHE["nc"] = build_module()
    nc = _NC_CACHE["nc"]

    in_maps = _host_prep(inputs, mask_np, kern, bias, chain, lb, rb)
    kw = {}
    if os.environ.get("KERNEL_TRACE_DIR"):
        kw["tmpdir"] = os.environ["KERNEL_TRACE_DIR"]
    res = run_bass_kernel_spmd(
        nc, in_maps, core_ids=list(range(NCORES)),
        trace=bool(int(os.environ.get("KERNEL_TRACE", "0"))),
        **kw,
    )
    pot = np.concatenate(
        [r["out_pot"].reshape(BL, T, U) for r in res.results], axis=0)
    oh = np.concatenate(
        [r["out_oh"].reshape(BL, T, U) for r in res.results], axis=0)
    if res.exec_time_ns is not None:
        print(f"HW exec time: {res.exec_time_ns} ns")
    return pot, oh

